# revision 32
# baseline (speedup 1.0000x reference)
"""Trainium2 Bass kernel for DenseMapsSoftmaxNeighborFinder.

reference:
    D2[i,j] = ||Y_i - X_j||^2
    P = softmax(-D2/tau, axis=-1)
    nn = argmax(P, -1)[:, None]          (== argmin D2, first-occurrence ties)
    dists = ||Y - P @ X||                (row 2-norm)
    returns (dists [Ny] f32, nn [Ny,1] i32)

Strategy (8 cores, shard Y rows, replicate X):
  Per core (q = 2048 queries, j = 16384 reference points, d = 256):
  - S[q,j] = Y.X - x^2/2 (row-max / argmax of S == argmin of D2; the y^2 row
    constant is irrelevant for softmax/argmax).
  - Pass A ([q-part, j-free] layout): fp32 PE matmuls with the -x^2/2 term
    folded in as extra contraction rows (bf16 triple-split for accuracy);
    per-chunk vector.max (top-8) + max_index give the exact row max M and
    global argmax.
  - Flash softmax fused into the same sweep (single S computation): per
    chunk, ACT computes w = exp((S - m_run)*2/tau) straight from the PSUM
    chunk (running-max shift as a per-partition bias), PE transposes the bf16
    weights to [j, q] layout, and PV accumulates acc += w^T.T @ [Xnat | 1]
    with an online rescale acc = acc*exp((m_old-m_new)*2/tau) + chunk_acc.
    The ones column gives l = sum_j w; P@X = acc/l; dists = ||Y - acc/l||.
Numerics: fp32 matmul error ~4e-5 << min top-2 D2 gap (6.4e-4 in this data),
so argmax matches the fp32 jax reference; softmax weights are fp32-accurate
(the bf16 rounding of the shift M cancels between numerator and denominator).
"""

import numpy as np

TAU = 0.07
D = 256
N_CORES = 8

_CACHE = {}
_PATCHED = False
_LAST_EXEC_NS = None


def _patch_tile_tail():
    """Work around walrus/bass ISA drift in this container.

    The stock Tile epilogue emits (a) an EVENT_SEMAPHORE_RANGE_CLEAR InstISA
    whose packed length this walrus rejects ("ISA wrong length") and (b) a
    Drain carrying ~27 semaphore waits ("Too many sync wait commands").
    Replace the tail with several small-wait nops (<=4 waits each) followed by
    a plain drain + barriers, and skip the semaphore range-clear.
    """
    global _PATCHED
    if _PATCHED:
        return
    import json

    import concourse.bass as bass
    import concourse.tile as tile
    from concourse.vector_clock import ScopedClock, VectorClock

    # This walrus supports few sem-wait slots per instruction.  Post-process
    # the BIR JSON: leave at most 1 wait on any non-NoOp instruction and hoist
    # the excess onto injected same-engine NoOps (<=4 waits each) just before.
    def _split_waits(js_bytes):
        js = json.loads(js_bytes)
        counter = [0]

        def fix_block(b):
            out = []
            for i in b.get("instructions", []):
                si = i.get("sync_info")
                w = (si or {}).get("on_wait") or []
                limit = 1
                if si and len(w) > limit:
                    excess, keep = w[:-limit], w[-limit:]
                    for g0 in range(0, len(excess), 1):
                        counter[0] += 1
                        out.append({
                            "debug": i.get("debug"),
                            "engine": i["engine"],
                            "ins": [],
                            "name": f"I-wsplit-{counter[0]}",
                            "opcode": "NoOp",
                            "outs": [],
                            "sync_info": {"on_update": [],
                                          "on_wait": excess[g0:g0 + 1]},
                        })
                    si["on_wait"] = keep
                out.append(i)
            b["instructions"] = out
            for sb in b.get("blocks", []):
                fix_block(sb)

        for f in js["functions"]:
            for b in f.get("blocks", []):
                fix_block(b)
        return json.dumps(js).encode()

    orig_tjb = bass.Bass.to_json_bytes

    def to_json_bytes(self, *a, **kw):
        return _split_waits(orig_tjb(self, *a, **kw))

    bass.Bass.to_json_bytes = to_json_bytes

    def _drain_and_barrier(self, tick_clock, wait_clock):
        gc = tick_clock.global_clock
        n = len(gc)
        CH = 4
        for base in range(0, n, CH):
            vals = [gc[p] if base <= p < base + CH else 0 for p in range(n)]
            if not any(vals):
                continue
            ni = self.nc.sync.nop()
            wait_clock.add_sem_waits(ni.ins, ScopedClock({None: VectorClock(vals)}))
        self.nc.sync.drain()
        self.nc.all_engine_barrier()
        popped = self.nc._tile_sem_poison_stack.pop()
        assert popped is self._sem_poison
        self.nc.all_engine_barrier()

    tile.TileContext._drain_and_barrier = _drain_and_barrier
    _PATCHED = True


def _build(nq, nj):
    """Build the Bass program for one core: nq queries vs nj reference points."""
    from contextlib import ExitStack

    import concourse.bass as bass
    import concourse.mybir as mybir
    import concourse.tile as tile
    from concourse.masks import make_identity

    _patch_tile_tail()

    f32 = mybir.dt.float32
    bf16 = mybir.dt.bfloat16
    i32 = mybir.dt.int32
    u32 = mybir.dt.uint32
    Alu = mybir.AluOpType
    Act = mybir.ActivationFunctionType

    assert nq % 128 == 0 and nj % 512 == 0
    n_tiles = nq // 128
    SC = 1024                      # pass-A chunk width (2 PSUM banks)
    n_sc = (nj + SC - 1) // SC
    n_jb = nj // 128               # 128-wide j blocks for pass B / PV
    GRP = 512                      # pass-B group width (4 j-blocks)
    n_grp = nj // GRP
    XW = 257                       # Xnat row stride: 256 dims + ones col

    nc = bass.Bass("TRN2", target_bir_lowering=False, debug=False)

    xt = nc.dram_tensor("xt", [D, nj], f32, kind="ExternalInput")
    xn = nc.dram_tensor("xn", [nj, D], f32, kind="ExternalInput")
    yt = nc.dram_tensor("yt", [D, nq], f32, kind="ExternalInput")
    yn = nc.dram_tensor("yn", [nq, D], f32, kind="ExternalInput")
    dists = nc.dram_tensor("dists", [nq], f32, kind="ExternalOutput")
    nnout = nc.dram_tensor("nn", [nq, 1], i32, kind="ExternalOutput")

    with tile.TileContext(nc) as tc, ExitStack() as ctx:
        dram = ctx.enter_context(tc.tile_pool(name="dram", bufs=1, space="DRAM"))
        consts = ctx.enter_context(tc.tile_pool(name="consts", bufs=1))

        # scratch[0:2,:] = 1.0 (pairs with -Mh,-Ml rows), scratch[2:5,:] =
        # bf16 triple split of -x^2/2 (pairs with ones).
        scratch = dram.tile([5, nj], bf16)

        # ---- resident tensors ----
        xt0 = consts.tile([128, nj], f32)
        xt1 = consts.tile([128, nj], f32)
        nc.sync.dma_start(xt0, xt[0:128, :])
        nc.sync.dma_start(xt1, xt[128:256, :])
        xnat = consts.tile([128, n_jb, XW], bf16)
        ones3 = consts.tile([3, 128], bf16)
        nc.vector.memset(ones3, 1.0)
        ident = consts.tile([128, 128], bf16)
        make_identity(nc, ident)
        rbuf = consts.tile([128, n_tiles], f32)
        nnbuf = consts.tile([128, n_tiles], i32)

        # ---- setup: Xnat bf16 cast, x^2, v-triple -> scratch ----
        nc.vector.memset(xnat[:, :, 256:257], 1.0)
        with tc.tile_pool(name="setup", bufs=2) as setup, \
                tc.tile_pool(name="setup1", bufs=1) as setup1, \
                tc.tile_pool(name="ps_mt", bufs=1, space="PSUM") as ps_mt:
            x2blk = setup1.tile([128, n_jb], f32)
            for jb in range(n_jb):
                blk = setup.tile([128, D], f32)
                nc.sync.dma_start(blk, xn[jb * 128:(jb + 1) * 128, :])
                nc.vector.tensor_copy(xnat[:, jb, 0:256], blk)
                sq = setup.tile([128, D], f32, tag="sq")
                nc.vector.tensor_mul(sq, blk, blk)
                nc.vector.tensor_reduce(out=x2blk[:, jb:jb + 1], in_=sq,
                                        axis=mybir.AxisListType.X, op=Alu.add)
            # v = -x^2/2 split into 3 bf16 planes (blocked [128, n_jb] layout)
            vcur = setup1.tile([128, n_jb], f32)
            nc.vector.tensor_scalar_mul(vcur, x2blk, -0.5)
            vback = setup1.tile([128, n_jb], f32)
            vpad = setup1.tile([128, 3, 128], bf16)
            vt_sb = setup1.tile([128, 3, 128], bf16)
            if n_jb < 128:
                nc.vector.memset(vpad, 0.0)
            for plane in range(3):
                nc.vector.tensor_copy(vpad[:, plane, 0:n_jb], vcur)  # cast bf16
                # residual for next plane: vcur -= fp32(bf16(vcur))
                if plane < 2:
                    nc.vector.tensor_copy(vback, vpad[:, plane, 0:n_jb])
                    nc.vector.tensor_tensor(out=vcur, in0=vcur, in1=vback,
                                            op=Alu.subtract)
                # transpose [128p, 128jb] -> [128jb, 128p], then row DMA out
                vt_ps = ps_mt.tile([128, 128], bf16, tag="mt")
                nc.tensor.transpose(vt_ps, vpad[:, plane, :], ident)
                nc.vector.tensor_copy(vt_sb[:, plane, :], vt_ps)
                nc.sync.dma_start(
                    scratch[2 + plane:3 + plane, :].rearrange(
                        "r (jb p) -> (r jb) p", p=128),
                    vt_sb[0:n_jb, plane, :])
            onesrow = setup1.tile([2, 512], bf16)
            nc.vector.memset(onesrow, 1.0)
            step = onesrow.shape[1]
            for c in range(0, nj, step):
                nc.sync.dma_start(scratch[0:2, c:c + step], onesrow)

        ytp = ctx.enter_context(tc.tile_pool(name="ytp", bufs=2))
        once = ctx.enter_context(tc.tile_pool(name="once", bufs=1))
        ch_p = ctx.enter_context(tc.tile_pool(name="ch", bufs=2))
        wsc_p = ctx.enter_context(tc.tile_pool(name="wsc", bufs=2))
        ps_a = ctx.enter_context(tc.tile_pool(name="ps_a", bufs=2, space="PSUM"))
        ps_wt = ctx.enter_context(tc.tile_pool(name="ps_wt", bufs=2, space="PSUM"))
        ps_acc = ctx.enter_context(tc.tile_pool(name="ps_acc", bufs=2, space="PSUM"))

        # ---- main loop over q tiles ----
        for t in range(n_tiles):
            qsl = slice(t * 128, (t + 1) * 128)
            yt0 = ytp.tile([128, 128], f32, tag="yt0")
            yt1 = ytp.tile([128, 128], f32, tag="yt1")
            nc.sync.dma_start(yt0, yt[0:128, qsl])
            nc.sync.dma_start(yt1, yt[128:256, qsl])

            mcbuf = once.tile([128, n_sc * 8], f32, tag="mcbuf")
            idxbuf = once.tile([128, n_sc * 8], f32, tag="idxbuf")
            mrun = once.tile([128, 1], f32, tag="mrun")
            nc.vector.memset(mrun, -1.0e6)
            macc = once.tile([128, XW], f32, tag="macc")
            nc.vector.memset(macc, 0.0)

            # ---- pass A: row max + argmax ----
            for sc in range(n_sc):
                w0 = sc * SC
                w1 = min(w0 + SC, nj)
                wid = w1 - w0
                psa = ps_a.tile([128, SC], f32, tag="psa")
                cha = ch_p.tile([3, SC], bf16, tag="ch")
                nc.gpsimd.dma_start(cha[:, 0:wid], scratch[2:5, w0:w1])
                for s0 in range(0, wid, 512):
                    s1 = min(s0 + 512, wid)
                    seg = slice(s0, s1)
                    jseg = slice(w0 + s0, w0 + s1)
                    nc.tensor.matmul(psa[:, seg], lhsT=yt0, rhs=xt0[:, jseg],
                                     start=True, stop=False)
                    nc.tensor.matmul(psa[:, seg], lhsT=yt1, rhs=xt1[:, jseg],
                                     start=False, stop=False)
                    nc.tensor.matmul(psa[:, seg], lhsT=ones3, rhs=cha[:, seg],
                                     start=False, stop=True)
                mc = mcbuf[:, sc * 8:(sc + 1) * 8]
                nc.vector.max(out=mc, in_=psa[:, 0:wid])
                idx8 = once.tile([128, 8], u32, tag="idx8")
                nc.vector.max_index(out=idx8, in_max=mc, in_values=psa[:, 0:wid])
                nc.vector.tensor_scalar_add(idxbuf[:, sc * 8:(sc + 1) * 8],
                                            idx8, float(w0))
                # flash-softmax: running max m, shift bias, w = exp((S-m)*2/tau)
                mprev = once.tile([128, 1], f32, tag="mprev")
                nc.vector.tensor_copy(mprev, mrun)
                nc.vector.tensor_max(mrun, mrun, mcbuf[:, sc * 8:sc * 8 + 1])
                negb = once.tile([128, 1], f32, tag="negb")
                nc.vector.tensor_scalar_mul(negb, mrun, -2.0 / TAU)
                alpha = once.tile([128, 1], f32, tag="alpha")
                nc.scalar.activation(alpha, mprev, func=Act.Exp, bias=negb,
                                     scale=2.0 / TAU)
                accps = ps_acc.tile([128, XW], f32, tag="accps")
                for half in range(SC // 512):
                    hsl = slice(half * 512, (half + 1) * 512)
                    wsc = wsc_p.tile([128, 512], bf16, tag="wsc")
                    nc.scalar.activation(wsc, psa[:, hsl], func=Act.Exp,
                                         bias=negb, scale=2.0 / TAU)
                    wt_ps = ps_wt.tile([128, 512], bf16, tag="wtps")
                    for jj in range(4):
                        nc.tensor.transpose(wt_ps[:, jj * 128:(jj + 1) * 128],
                                            wsc[:, jj * 128:(jj + 1) * 128], ident)
                    wt_sb = wsc_p.tile([128, 512], bf16, tag="wtsb")
                    nc.scalar.activation(wt_sb, wt_ps, func=Act.Copy, bias=0.0,
                                         scale=1.0)
                    for jj in range(4):
                        jb = sc * 8 + half * 4 + jj
                        nc.tensor.matmul(accps,
                                         lhsT=wt_sb[:, jj * 128:(jj + 1) * 128],
                                         rhs=xnat[:, jb, :],
                                         start=(half == 0 and jj == 0),
                                         stop=(half == 1 and jj == 3))
                # acc = acc*alpha + sum_{j in SC} w_j * [X | 1]
                nc.vector.scalar_tensor_tensor(out=macc, in0=macc, scalar=alpha,
                                               in1=accps, op0=Alu.mult, op1=Alu.add)

            # combine: global max M + first-occurrence argmax
            gtop = once.tile([128, 8], f32, tag="gtop")
            nc.vector.max(out=gtop, in_=mcbuf)
            mval = gtop[:, 0:1]
            eq = once.tile([128, n_sc * 8], f32, tag="eq")
            nc.vector.tensor_scalar(eq, mcbuf, scalar1=mval, scalar2=None,
                                    op0=Alu.is_equal)
            # cand = idx + (1-eq)*2^24 ; min over candidates
            nc.vector.tensor_scalar(eq, eq, scalar1=-16777216.0, scalar2=16777216.0,
                                    op0=Alu.mult, op1=Alu.add)
            nc.vector.tensor_tensor(out=eq, in0=idxbuf, in1=eq, op=Alu.add)
            nnf = once.tile([128, 1], f32, tag="nnf")
            nc.vector.tensor_reduce(out=nnf, in_=eq, axis=mybir.AxisListType.X,
                                    op=Alu.min)
            nc.vector.tensor_copy(nnbuf[:, t:t + 1], nnf)

            # ---- flash epilogue state is updated inside the SC loop above ----
            linv = once.tile([128, 1], f32, tag="linv")
            nc.vector.reciprocal(linv, macc[:, 256:257])
            nlinv = once.tile([128, 1], f32, tag="nlinv")
            nc.vector.tensor_scalar_mul(nlinv, linv, -1.0)
            ynat = once.tile([128, D], f32, tag="ynat")
            nc.sync.dma_start(ynat, yn[qsl, :])
            diff = once.tile([128, D], f32, tag="diff")
            nc.vector.scalar_tensor_tensor(out=diff, in0=macc[:, 0:256],
                                           scalar=nlinv, in1=ynat,
                                           op0=Alu.mult, op1=Alu.add)
            nc.vector.tensor_mul(diff, diff, diff)
            nc.vector.tensor_reduce(out=rbuf[:, t:t + 1], in_=diff,
                                    axis=mybir.AxisListType.X, op=Alu.add)

        # ---- finalize ----
        drow = consts.tile([128, n_tiles], f32)
        nc.scalar.activation(drow, rbuf, func=Act.Sqrt, bias=0.0, scale=1.0)
        nc.sync.dma_start(dists.ap().rearrange("(t p) -> p t", p=128), drow)
        nc.sync.dma_start(nnout.ap().rearrange("(t p) one -> p (t one)", p=128),
                          nnbuf)
    return nc


def _get_nc(nq, nj):
    key = (nq, nj)
    if key not in _CACHE:
        _CACHE[key] = _build(nq, nj)
    return _CACHE[key]


def kernel(X: np.ndarray, Y: np.ndarray):
    from concourse import bass_utils

    nj, d = X.shape
    nq_total = Y.shape[0]
    assert d == D and nq_total % N_CORES == 0
    nq = nq_total // N_CORES

    nc = _get_nc(nq, nj)

    Xc = np.ascontiguousarray(X, dtype=np.float32)
    XT = np.ascontiguousarray(Xc.T)
    in_maps = []
    for c in range(N_CORES):
        Yc = np.ascontiguousarray(Y[c * nq:(c + 1) * nq], dtype=np.float32)
        in_maps.append({
            "xt": XT,
            "xn": Xc,
            "yt": np.ascontiguousarray(Yc.T),
            "yn": Yc,
        })
    res = bass_utils.run_bass_kernel_spmd(nc, in_maps, core_ids=list(range(N_CORES)))
    global _LAST_EXEC_NS
    if res.exec_time_ns is not None:
        _LAST_EXEC_NS = res.exec_time_ns
    dists = np.concatenate([r["dists"] for r in res.results])
    nn = np.concatenate([r["nn"] for r in res.results]).astype(np.int32)
    return dists, nn
